# revision 1
# baseline (speedup 1.0000x reference)
"""Cubic-Bezier Gaussian rasterizer for Trainium2 (Bass/Tile), 8-core SPMD.

Math (matches reference.py):
    t = linspace(0, 1, 100);  curve = Bezier3(control_points, t)   # (2, 100)
    gx[t, i] = exp(-(curve_x[t] - i/8192)^2 / 2e-4)                # (100, 8192)
    gy[t, j] = exp(-(curve_y[t] - j/8192)^2 / 2e-4)
    out = gx^T @ gy / 100                                          # (8192, 8192)

Sharding: output rows across 8 cores. Each core computes gx for its 1024
grid-row values, the full gy, and a local (1024 x 8192) matmul. No
communication; host concatenates the row slices.

Device pipeline per core (the only DMA traffic is one 2 KB input and the
32 MB output, which is the memory-regime floor):
  PE:   negc = [neg_basis; 1]^T @ [cp; rowoff] (per-core row offset folded
        into a 5th contraction row), then 128 f32r matmuls gx^T @ gy -> PSUM
  Pool: one 1024-wide iota generates the grid ramp on-chip (exact in f32);
        each chunk's column offset is folded into its Square bias
  ACT:  Square/Exp Gaussian tables (squares alternate with DVE) + ~2/5 of
        the PSUM->SBUF copies
  DVE:  the other squares + most PSUM->SBUF copies
  DMA:  column-major 512 KB stores, issued per (row-block, column) tile so
        the DMA engines saturate right after the first gy chunk

Timing (TimelineSim cost model, cross-checked on hardware by slope-fitting
wall time over an in-kernel repetition loop): ~104.3 us per core
end-to-end (8.3 us pipeline fill + 94.3 us saturated output stream +
1.6 us drain); measured steady-state pass 102-107 us including ~4 us
loop overhead. The stream runs at ~343 GB/s effective per core with all
8 cores writing concurrently, ~95% of the per-NeuronCore HBM bound.
"""

import math
import os

import numpy as np

RES = 8192
STEPS = 100
N_CORES = 8
ROWS_PER_CORE = RES // N_CORES  # 1024
NEG_INV_2SIG = -5000.0  # -1 / 0.0002
LN_INV_STEPS = float(np.log(np.float64(1.0) / STEPS))

M_TILE = 128  # output rows per PE matmul (psum partition dim)
MM_N = 512  # matmul moving free dim (one PSUM bank of f32)
PS_COLS = 1024  # psum tile free size (2 banks -> 2 matmuls per copy)
GY_CHUNK = 1024  # max gy chunk size for square/exp ops
# First chunks are narrow so the very first stores launch earlier; the
# rest use the full width. Must sum to RES.
GY_WIDTHS = [512, 512] + [1024] * 7
GY_OFFS = [sum(GY_WIDTHS[:i]) for i in range(len(GY_WIDTHS))]
N_GY = len(GY_WIDTHS)

# "f32"  : exact fp32 matmul, 4 cycles/row on the PE
# "f32r" : single-pass fp32 matmul, 1 cycle/row (relaxed multiply precision)
MM_MODE = os.environ.get("BEZ_MM_MODE", "f32r")

_CACHE = {}


def _build_nc(mm_mode=None, reps=1):
    import concourse.mybir as mybir
    import concourse.tile as tile
    from concourse import bacc

    if mm_mode is None:
        mm_mode = MM_MODE
    f32 = mybir.dt.float32
    f32r = mybir.dt.float32r
    nc = bacc.Bacc(
        "TRN2", target_bir_lowering=False, debug=False, num_devices=N_CORES
    )

    # Single tiny input: [:, :100] = [neg_basis; ones] (4+1 x 100),
    # [:, 100:102] = [control_points; [row_offset, 0]] (4+1 x 2).
    comb_d = nc.dram_tensor("curve_in", [5, STEPS + 2], f32, kind="ExternalInput")
    out_d = nc.dram_tensor("out", [ROWS_PER_CORE, RES], f32, kind="ExternalOutput")

    m_tiles = ROWS_PER_CORE // M_TILE  # 8

    exp = mybir.ActivationFunctionType.Exp
    square = mybir.ActivationFunctionType.Square
    add = mybir.AluOpType.add
    mult = mybir.AluOpType.mult

    g_dt = f32r if mm_mode == "f32r" else f32

    with tile.TileContext(nc) as tc:
        with (
            tc.tile_pool(name="const", bufs=1) as const,
            tc.tile_pool(name="gyp", bufs=N_GY) as gyp,
            tc.tile_pool(name="stage", bufs=4) as stage,
            tc.tile_pool(name="obuf", bufs=8) as obuf,
            tc.tile_pool(name="psmm", bufs=3, space="PSUM") as psmm,
            tc.tile_pool(name="pscurve", bufs=1, space="PSUM") as pscurve,
        ):
            # t=0: preload the ACT Exp/Square/Copy table via a dummy op.
            lnbias = const.tile([STEPS, 1], f32)
            nc.vector.memset(lnbias, LN_INV_STEPS)
            inv_res = const.tile([STEPS, 1], f32)
            nc.vector.memset(inv_res, 1.0 / RES)
            actwarm = const.tile([STEPS, 1], f32)
            nc.scalar.activation(out=actwarm, in_=lnbias, func=exp)

            # One shared grid ramp: iota_t[t, i] = i exactly in f32. Each gy
            # chunk's column offset is folded into its per-partition Square
            # bias below, so a single 1024-wide iota serves all chunks.
            iota_t = const.tile([STEPS, GY_CHUNK], f32)
            nc.gpsimd.iota(
                iota_t,
                pattern=[[1, GY_CHUNK]],
                base=0,
                channel_multiplier=0,
                allow_small_or_imprecise_dtypes=True,
            )

            # biases[:, g] = chunk_offset/RES (memset now) + negc_y (added
            # once the curve matmul lands).
            biases = const.tile([STEPS, N_GY], f32)
            for g in range(N_GY):
                nc.vector.memset(biases[:, g : g + 1], GY_OFFS[g] / RES)

            # negc[t] = (-cx[t] + rowoff, -cy[t]): one DMA + one K=5 matmul.
            comb = const.tile([5, STEPS + 2], f32)
            nc.sync.dma_start(out=comb, in_=comb_d.ap())
            negc_ps = pscurve.tile([STEPS, 2], f32)
            nc.tensor.matmul(
                out=negc_ps,
                lhsT=comb[:, :STEPS],
                rhs=comb[:, STEPS : STEPS + 2],
                start=True,
                stop=True,
            )
            negc = const.tile([STEPS, 2], f32)
            nc.vector.tensor_copy(out=negc, in_=negc_ps)
            nc.vector.tensor_scalar(
                out=biases,
                in0=biases,
                scalar1=negc[:, 1:2],
                scalar2=None,
                op0=add,
            )

            # gx = exp(-5000*(rowoff + i/8192 - cx)^2 + ln(1/100)), split so
            # the first output tile (row-block 0, needing only columns
            # 0..127) isn't gated on the full-width chain: gxa (128 cols,
            # ACT, ~0.6us) unblocks the first store; gxb (896 cols, DVE
            # square) computes while the first stores already stream out.
            gxa_s = stage.tile([STEPS, M_TILE], f32, tag="gys")
            nc.scalar.activation(
                out=gxa_s,
                in_=iota_t[:, :M_TILE],
                func=square,
                scale=1.0 / RES,
                bias=negc[:, 0:1],
            )
            gxa = const.tile([STEPS, M_TILE], g_dt)
            nc.scalar.activation(
                out=gxa, in_=gxa_s, func=exp, scale=NEG_INV_2SIG, bias=lnbias
            )

            gxb = None  # emitted after the first column tile, see below

            def emit_gxb():
                gxb_s = stage.tile([STEPS, ROWS_PER_CORE - M_TILE], f32, tag="gys")
                nc.vector.tensor_scalar(
                    out=gxb_s,
                    in0=iota_t[:, M_TILE:ROWS_PER_CORE],
                    scalar1=inv_res,
                    scalar2=negc[:, 0:1],
                    op0=mult,
                    op1=add,
                )
                nc.vector.tensor_mul(out=gxb_s, in0=gxb_s, in1=gxb_s)
                t = const.tile([STEPS, ROWS_PER_CORE - M_TILE], g_dt)
                nc.scalar.activation(
                    out=t, in_=gxb_s, func=exp, scale=NEG_INV_2SIG, bias=lnbias
                )
                return t

            gy_chunks = [None] * N_GY
            copy_state = [0]

            def emit_gy_chunk(g):
                # gy chunk g = exp(-5000*((i + off_g)/8192 - cy)^2), read
                # from the shared ramp with the chunk offset folded into the
                # bias; squares alternate ACT/DVE to balance engines.
                w = GY_WIDTHS[g]
                gys = stage.tile([STEPS, w], f32, tag="gys")
                if g % 2 == 0:
                    nc.scalar.activation(
                        out=gys,
                        in_=iota_t[:, :w],
                        func=square,
                        scale=1.0 / RES,
                        bias=biases[:, g : g + 1],
                    )
                else:
                    nc.vector.tensor_scalar(
                        out=gys,
                        in0=iota_t[:, :w],
                        scalar1=inv_res,
                        scalar2=biases[:, g : g + 1],
                        op0=mult,
                        op1=add,
                    )
                    nc.vector.tensor_mul(out=gys, in0=gys, in1=gys)
                gyc = gyp.tile([STEPS, w], g_dt, tag="gyc")
                nc.scalar.activation(out=gyc, in_=gys, func=exp, scale=NEG_INV_2SIG)
                gy_chunks[g] = gyc

            def emit_col_tile(mi, g):
                # one (row-block, column-chunk) tile: 1-2 matmuls -> PSUM,
                # one PSUM->SBUF copy, one 256-512 KB store.
                row0 = mi * M_TILE
                col0 = GY_OFFS[g]
                w = GY_WIDTHS[g]
                gyc = gy_chunks[g]
                lhsT = (
                    gxa if mi == 0 else gxb[:, row0 - M_TILE : row0]
                )
                ps = psmm.tile([M_TILE, w], f32, tag="ps")
                for h in range(0, w, MM_N):
                    hw = min(MM_N, w - h)
                    nc.tensor.matmul(
                        out=ps[:, h : h + hw],
                        lhsT=lhsT,
                        rhs=gyc[:, h : h + hw],
                        start=True,
                        stop=True,
                    )
                ob = obuf.tile([M_TILE, w], f32, tag="ob")
                # PSUM->SBUF copies alternate ACT/DVE evenly
                if copy_state[0] % 2 == 1:
                    nc.scalar.copy(out=ob, in_=ps)
                else:
                    nc.vector.tensor_copy(out=ob, in_=ps)
                copy_state[0] += 1
                nc.sync.dma_start(
                    out=out_d.ap()[row0 : row0 + M_TILE, col0 : col0 + w],
                    in_=ob,
                )

            # --- main loop, column-major: as each gy chunk lands, all 8
            # row-blocks' matmuls for that column run and their 512 KB
            # tiles stream straight out. The DMA engines saturate right
            # after the first chunk and never wait on a row-block assembly.
            # (reps>1 wraps the steady state in a dynamic loop, for
            # benchmarking only.)
            if reps == 1:
                emit_gy_chunk(0)
                emit_col_tile(0, 0)  # first store: gxa + chunk 0 only
                # overlaps the first store; deprioritized so the scheduler
                # doesn't slot its DVE square into the chunk-0 chain
                with tc.high_priority(-12):
                    gxb = emit_gxb()
                for mi in range(1, m_tiles):
                    emit_col_tile(mi, 0)
                for g in range(1, N_GY):
                    emit_gy_chunk(g)
                    for mi in range(m_tiles):
                        emit_col_tile(mi, g)
            else:
                gxb = emit_gxb()
                for g in range(N_GY):
                    emit_gy_chunk(g)
                with tc.For_i(0, reps, 1, hint_engines=(mybir.EngineType.PE,)):
                    for g in range(N_GY):
                        for mi in range(m_tiles):
                            emit_col_tile(mi, g)

    nc.compile()
    return nc


def _get_nc():
    if "nc" not in _CACHE:
        _CACHE["nc"] = _build_nc()
    return _CACHE["nc"]


def _host_constants():
    if "consts" not in _CACHE:
        t = np.linspace(0.0, 1.0, STEPS, dtype=np.float32).astype(np.float64)
        basis = np.stack(
            [math.comb(3, k) * (1.0 - t) ** (3 - k) * t**k for k in range(4)]
        )  # (4, STEPS) float64
        nb5 = np.concatenate(
            [-basis, np.ones((1, STEPS), np.float64)], axis=0
        ).astype(np.float32)  # (5, STEPS): [-basis; ones]
        _CACHE["consts"] = nb5
    return _CACHE["consts"]


TRACE = False
LAST_RESULT = None


def kernel(control_points: np.ndarray) -> np.ndarray:
    global LAST_RESULT
    from concourse.bass_utils import run_bass_kernel_spmd

    nc = _get_nc()
    nb5 = _host_constants()
    cp = np.ascontiguousarray(np.asarray(control_points), dtype=np.float32)

    in_maps = []
    for c in range(N_CORES):
        rowoff = np.float32(c * ROWS_PER_CORE) / np.float32(RES)
        cp5 = np.concatenate(
            [cp, np.array([[rowoff, 0.0]], np.float32)], axis=0
        )  # (5, 2)
        comb = np.concatenate([nb5, cp5], axis=1)  # (5, 102)
        in_maps.append({"curve_in": np.ascontiguousarray(comb)})

    res = run_bass_kernel_spmd(
        nc, in_maps, core_ids=list(range(N_CORES)), trace=TRACE
    )
    LAST_RESULT = res
    return np.concatenate([res.results[c]["out"] for c in range(N_CORES)], axis=0)



# revision 5
# speedup vs baseline: 4.4472x; 4.4472x over previous
"""Cubic-Bezier Gaussian rasterizer for Trainium2 (Bass/Tile), 8-core SPMD.

Math (matches the reference):
    t = linspace(0, 1, 100);  curve = Bezier3(control_points, t)   # (2, 100)
    gx[t, i] = exp(-(curve_x[t] - i/8192)^2 / 2e-4)                # (100, 8192)
    gy[t, j] = exp(-(curve_y[t] - j/8192)^2 / 2e-4)
    out = gx^T @ gy / 100                                          # (8192, 8192)

The Gaussian tube around the curve (sigma = 0.01 = ~82 px) covers only
~15% of the 8192 x 8192 image at 128 x 512 tile granularity; everything
else is < 1e-6 (vs a Frobenius norm of ~27), far below the accuracy that
fp16 storage already caps. So instead of streaming the full 256 MB f32
image (the baseline, ~104 us at the 360 GB/s DMA roofline), the host
plans the active tile set from the curve at call time and the device
computes just those tiles, in fp16. The host scatters them into a zero
image while unsharding.

Plan (host, numpy, per call; compiled program cached by shape (C, R)):
  - active tiles: dist(tile rect, curve point) bound per (row-block,
    col-chunk); threshold 1e-7 of the summed Gaussian.
  - cover each chunk's active row-blocks with windows of R consecutive
    row-blocks; pad the window list to 8*C windows (repeats are benign:
    duplicate slots just rewrite the same correct tile).
  - per-window data = two Gaussian-center bias vectors (100 floats each):
    sqrt(5000)*(chunk_px/8192 - cy[t]) and sqrt(5000)*(win_px/8192 - cx[t]).

Device pipeline per core (C groups x R slots):
  ACT:  one Derivative_Erf per gy chunk [100,512] and one per gx window
        [100,128R] - Derivative_Erf(u) = (2/sqrt(pi)) exp(-u^2) is a
        single-instruction Gaussian (verified exact to 2e-6 on TRN2);
        the (2/sqrt(pi))^2 and the 1/100 fold into the copy scale.
  PE:   R f32r matmuls per group, gxw^T @ gy -> PSUM in [128,1024] pairs
  DVE/Pool/ACT: paired PSUM->SBUF copies, scale pi/400, downcast to fp16
        (engine chosen greedily to balance modeled busy time)
  DMA:  2 stores per group (fp16, 2 KB lines), ~2.6 MB per core total
"""

import math

import numpy as np

RES = 8192
STEPS = 100
N_CORES = 8
TWO_SIGMA_SQ = 2e-4
K_GAUSS = math.sqrt(1.0 / TWO_SIGMA_SQ)  # sqrt(5000)
COPY_SCALE = math.pi / 4.0 / STEPS  # undo (2/sqrt(pi))^2, apply 1/STEPS

TILE_P = 128  # output tile rows (psum partition dim)
TILE_F = 512  # output tile cols (one psum bank of f32)
N_RB = RES // TILE_P  # 64 row-blocks
N_CH = RES // TILE_F  # 16 column-chunks
ACT_THR = 1e-7  # tile activity threshold on the summed-Gaussian bound

_CACHE = {}

TRACE = False
LAST_RESULT = None


# ----------------------------------------------------------------- planning


def _bezier_xy(cp):
    """Cubic Bezier samples, float64, shape (2, STEPS)."""
    t = np.linspace(0.0, 1.0, STEPS)
    b = np.stack(
        [math.comb(3, k) * (1.0 - t) ** (3 - k) * t**k for k in range(4)]
    )  # (4, STEPS)
    return cp.astype(np.float64).T @ b  # (2, STEPS)


def _active_tiles(cx, cy):
    """Bool (N_RB, N_CH): tiles where the summed Gaussian can exceed ACT_THR."""
    rb_lo = np.arange(N_RB) * TILE_P / RES
    rb_hi = (np.arange(N_RB) * TILE_P + (TILE_P - 1)) / RES
    ch_lo = np.arange(N_CH) * TILE_F / RES
    ch_hi = (np.arange(N_CH) * TILE_F + (TILE_F - 1)) / RES
    # distance from each curve point to each tile interval (0 if inside)
    dx = np.maximum(0.0, np.maximum(rb_lo[:, None] - cx, cx - rb_hi[:, None]))
    dy = np.maximum(0.0, np.maximum(ch_lo[:, None] - cy, cy - ch_hi[:, None]))
    # upper bound of the tile max: each step evaluated at its closest point
    d2 = dx[:, None, :] ** 2 + dy[None, :, :] ** 2  # (N_RB, N_CH, STEPS)
    bound = np.exp(-d2 / TWO_SIGMA_SQ).sum(-1) / STEPS
    return bound > ACT_THR


def _windows_for(active, R):
    """Greedy cover of each chunk's active row-blocks with windows of R
    consecutive row-blocks (bridges gaps < R). Returns list of
    (chunk, rb_start)."""
    wins = []
    for ch in range(N_CH):
        rbs = np.nonzero(active[:, ch])[0]
        i = 0
        while i < len(rbs):
            start = min(int(rbs[i]), N_RB - R)
            wins.append((ch, max(0, start)))
            while i < len(rbs) and rbs[i] < start + R:
                i += 1
    return wins


# modeled busy us per engine (Pool can't do tensor ops on this compiler):
# (pair copy [128,1024], single copy [128,512])
_COPY_COST = {
    "dve": (1.192, 0.658),
    "act": (1.038, 0.546),
}


def _plan_shape(active):
    """Pick (R, windows) minimizing a coarse makespan model."""
    best = None
    for R in range(3, 11):
        wins = _windows_for(active, R)
        C = max(1, (len(wins) + N_CORES - 1) // N_CORES)
        S = C * R
        dma = S * 0.364 + 1.0
        act = C * (0.612 + 0.107 * R + 0.185)
        busy = {"dve": 0.0, "act": act}
        for _ in range(C):
            for c in range(R // 2):
                eng = min(busy, key=lambda e: busy[e] + _COPY_COST[e][0])
                busy[eng] += _COPY_COST[eng][0]
            if R % 2:
                eng = min(busy, key=lambda e: busy[e] + _COPY_COST[e][1])
                busy[eng] += _COPY_COST[eng][1]
        score = max(dma, max(busy.values()) + 1.0, S * 0.30)
        if best is None or score < best[0]:
            best = (score, R, wins)
    return best[1], best[2]


# ------------------------------------------------------------- device build


def _build_nc(C, R):
    import concourse.mybir as mybir
    import concourse.tile as tile
    from concourse import bacc

    f32 = mybir.dt.float32
    f32r = mybir.dt.float32r
    f16 = mybir.dt.float16
    derf = mybir.ActivationFunctionType.Derivative_Erf
    cpy = mybir.ActivationFunctionType.Copy
    mult = mybir.AluOpType.mult

    nc = bacc.Bacc(
        "TRN2", target_bir_lowering=False, debug=False, num_devices=N_CORES
    )

    # bias[:, g]     = K*(chunk_px/RES - cy[t])   for group g
    # bias[:, C+g]   = K*(win_px/RES - cx[t])     for group g
    bias_d = nc.dram_tensor("bias_in", [STEPS, 2 * C], f32, kind="ExternalInput")
    # per-core output, slot-major columns: out[p, (g*R+r)*TILE_F + c]
    out_d = nc.dram_tensor("out", [TILE_P, C * R * TILE_F], f16, kind="ExternalOutput")

    WIOTA = max(TILE_F, TILE_P * R)
    n_pairs = (R + 1) // 2

    # greedy copy-engine balancing against modeled busy time
    busy = {"dve": 0.0, "act": C * (0.612 + 0.107 * R + 0.185)}

    with tile.TileContext(nc) as tc:
        with (
            tc.tile_pool(name="const", bufs=1) as const,
            tc.tile_pool(name="gyp", bufs=3) as gyp,
            tc.tile_pool(name="gxp", bufs=3) as gxp,
            tc.tile_pool(name="obuf", bufs=3) as obuf,
            tc.tile_pool(name="psmm", bufs=4, space="PSUM") as psmm,
        ):
            # ACT table warm-up (overlaps the bias DMA latency)
            warm = const.tile([STEPS, 1], f32)
            nc.vector.memset(warm, 0.0)
            warm2 = const.tile([STEPS, 1], f32)
            nc.scalar.activation(out=warm2, in_=warm, func=derf)

            iota_t = const.tile([STEPS, WIOTA], f32)
            nc.gpsimd.iota(
                iota_t,
                pattern=[[1, WIOTA]],
                base=0,
                channel_multiplier=0,
                allow_small_or_imprecise_dtypes=True,
            )

            bias_t = const.tile([STEPS, 2 * C], f32)
            nc.sync.dma_start(out=bias_t, in_=bias_d.ap())

            def copy_op(out_ap, in_ap, is_pair):
                pair, single = (1.0, 0.0) if is_pair else (0.0, 1.0)
                eng = min(
                    busy,
                    key=lambda e: busy[e]
                    + _COPY_COST[e][0] * pair
                    + _COPY_COST[e][1] * single,
                )
                busy[eng] += _COPY_COST[eng][0] * pair + _COPY_COST[eng][1] * single
                if eng == "act":
                    nc.scalar.activation(
                        out=out_ap, in_=in_ap, func=cpy, scale=COPY_SCALE
                    )
                else:
                    nc.vector.tensor_scalar(
                        out=out_ap, in0=in_ap, scalar1=COPY_SCALE, scalar2=None,
                        op0=mult,
                    )

            for g in range(C):
                # single-instruction Gaussians: DerivErf(K/RES * i + bias)
                gy = gyp.tile([STEPS, TILE_F], f32r, tag="gy")
                nc.scalar.activation(
                    out=gy,
                    in_=iota_t[:, :TILE_F],
                    func=derf,
                    scale=K_GAUSS / RES,
                    bias=bias_t[:, g : g + 1],
                )
                gxw = gxp.tile([STEPS, TILE_P * R], f32r, tag="gx")
                nc.scalar.activation(
                    out=gxw,
                    in_=iota_t[:, : TILE_P * R],
                    func=derf,
                    scale=K_GAUSS / RES,
                    bias=bias_t[:, C + g : C + g + 1],
                )
                ob = obuf.tile([TILE_P, R * TILE_F], f16, tag="ob")
                for p in range(n_pairs):
                    r0 = 2 * p
                    w = min(2, R - r0)  # 2, or 1 for the odd tail
                    ps = psmm.tile([TILE_P, 1024], f32, tag="ps")
                    for r in (r0, r0 + 1)[:w]:
                        nc.tensor.matmul(
                            out=ps[:, (r - r0) * TILE_F : (r - r0 + 1) * TILE_F],
                            lhsT=gxw[:, r * TILE_P : (r + 1) * TILE_P],
                            rhs=gy,
                            start=True,
                            stop=True,
                        )
                    copy_op(
                        ob[:, r0 * TILE_F : (r0 + w) * TILE_F],
                        ps[:, : w * TILE_F],
                        is_pair=(w == 2),
                    )
                    # two stores per group: after the first pair, and after
                    # the rest (keeps HWDGE count at 2C, lines at 2 KB)
                    if p == 0 and n_pairs > 1:
                        nc.sync.dma_start(
                            out=out_d.ap()[
                                :, (g * R) * TILE_F : (g * R + 2) * TILE_F
                            ],
                            in_=ob[:, : 2 * TILE_F],
                        )
                    elif p == n_pairs - 1:
                        lo = 2 if n_pairs > 1 else 0
                        nc.sync.dma_start(
                            out=out_d.ap()[
                                :, (g * R + lo) * TILE_F : (g * R + R) * TILE_F
                            ],
                            in_=ob[:, lo * TILE_F : R * TILE_F],
                        )

    nc.compile()
    return nc


def _get_nc():
    return _CACHE["nc"]


# ------------------------------------------------------------------ kernel


def kernel(control_points: np.ndarray) -> np.ndarray:
    global LAST_RESULT
    from concourse.bass_utils import run_bass_kernel_spmd

    cp = np.asarray(control_points, dtype=np.float32)
    cx, cy = _bezier_xy(cp)

    active = _active_tiles(cx, cy)
    R, wins = _plan_shape(active)
    C = max(1, (len(wins) + N_CORES - 1) // N_CORES)
    wins = wins + [wins[-1]] * (N_CORES * C - len(wins))  # pad with repeats

    key = (C, R)
    if key not in _CACHE or _CACHE.get("key") != key:
        _CACHE["nc"] = _build_nc(C, R)
        _CACHE["key"] = key
    nc = _CACHE["nc"]

    cxf = cx.astype(np.float32)
    cyf = cy.astype(np.float32)
    in_maps = []
    per_core = [wins[c * C : (c + 1) * C] for c in range(N_CORES)]
    for c in range(N_CORES):
        bias = np.empty((STEPS, 2 * C), np.float32)
        for g, (ch, rb0) in enumerate(per_core[c]):
            bias[:, g] = np.float32(K_GAUSS) * (
                np.float32(ch * TILE_F) / np.float32(RES) - cyf
            )
            bias[:, C + g] = np.float32(K_GAUSS) * (
                np.float32(rb0 * TILE_P) / np.float32(RES) - cxf
            )
        in_maps.append({"bias_in": np.ascontiguousarray(bias)})

    res = run_bass_kernel_spmd(
        nc, in_maps, core_ids=list(range(N_CORES)), trace=TRACE
    )
    LAST_RESULT = res

    img = np.zeros((RES, RES), np.float32)
    for c in range(N_CORES):
        arr = res.results[c]["out"]  # (TILE_P, C*R*TILE_F) fp16
        slots = (
            arr.reshape(TILE_P, C * R, TILE_F)
            .transpose(1, 0, 2)
            .astype(np.float32)
        )
        for g, (ch, rb0) in enumerate(per_core[c]):
            for r in range(R):
                rb = rb0 + r
                img[
                    rb * TILE_P : (rb + 1) * TILE_P,
                    ch * TILE_F : (ch + 1) * TILE_F,
                ] = slots[g * R + r]
    return img


# revision 6
# speedup vs baseline: 4.5249x; 1.0175x over previous
"""Cubic-Bezier Gaussian rasterizer for Trainium2 (Bass/Tile), 8-core SPMD.

Math (matches the reference):
    t = linspace(0, 1, 100);  curve = Bezier3(control_points, t)   # (2, 100)
    gx[t, i] = exp(-(curve_x[t] - i/8192)^2 / 2e-4)                # (100, 8192)
    gy[t, j] = exp(-(curve_y[t] - j/8192)^2 / 2e-4)
    out = gx^T @ gy / 100                                          # (8192, 8192)

The Gaussian tube around the curve (sigma = 0.01 = ~82 px) covers only
~15% of the 8192 x 8192 image at 128 x 512 tile granularity; everything
else is < 1e-6 (vs a Frobenius norm of ~27), far below the error that
fp16 storage already introduces (2e-4). So instead of streaming the full
256 MB f32 image (the baseline, ~104 us at the 360 GB/s DMA roofline),
the host plans the active tile set from the curve at call time and the
device computes just those tiles, in fp16. The host scatters them into a
zero image while unsharding.

Plan (host, numpy, per call; compiled program cached by shape (C, R)):
  - active tiles: dist(tile rect, curve point) bound per (row-block,
    col-chunk); threshold 1e-7 on the summed Gaussian.
  - cover each chunk's active row-blocks with windows of R consecutive
    row-blocks (greedy, bridges small gaps); pad the window list to
    8*C windows (repeats are benign: duplicate slots just rewrite the
    same correct tile).
  - per-window data = two Gaussian-center bias vectors (100 floats):
    sqrt(5000)*(chunk_px/8192 - cy[t]) and sqrt(5000)*(win_px/8192 - cx[t]).

Device pipeline per core (C groups x R slots, R even):
  ACT:  one Derivative_Erf per gy chunk [100,512] and per gx window
        [100,128R] - Derivative_Erf(u) = (2/sqrt(pi)) exp(-u^2) is a
        single-instruction Gaussian (exact to 2e-6 on TRN2); the
        (2/sqrt(pi))^2 and the 1/100 fold into the copy scale.
  PE:   R f32r matmuls per group, gxw^T @ gy -> PSUM in [128,1024] pairs
  ACT/DVE: per-pair PSUM->SBUF copies, scale pi/400, downcast to fp16
        (engine chosen greedily to balance modeled busy time; the Pool
        engine cannot run tensor ops on this compiler)
  DMA:  one store per pair (256 KB fp16, 2 KB lines), alternating
        SP/HWDGE and Pool/SWDGE issue queues; ~3 MB per core total
"""

import math

import numpy as np

RES = 8192
STEPS = 100
N_CORES = 8
TWO_SIGMA_SQ = 2e-4
K_GAUSS = math.sqrt(1.0 / TWO_SIGMA_SQ)  # sqrt(5000)
COPY_SCALE = math.pi / 4.0 / STEPS  # undo (2/sqrt(pi))^2, apply 1/STEPS

TILE_P = 128  # output tile rows (psum partition dim)
TILE_F = 512  # output tile cols (one psum bank of f32)
N_RB = RES // TILE_P  # 64 row-blocks
N_CH = RES // TILE_F  # 16 column-chunks
ACT_THR = 1e-7  # tile activity threshold on the summed-Gaussian bound

_CACHE = {}

TRACE = False
LAST_RESULT = None


# ----------------------------------------------------------------- planning


def _bezier_xy(cp):
    """Cubic Bezier samples, float64, shape (2, STEPS)."""
    t = np.linspace(0.0, 1.0, STEPS)
    b = np.stack(
        [math.comb(3, k) * (1.0 - t) ** (3 - k) * t**k for k in range(4)]
    )  # (4, STEPS)
    return cp.astype(np.float64).T @ b  # (2, STEPS)


def _active_tiles(cx, cy):
    """Bool (N_RB, N_CH): tiles where the summed Gaussian can exceed ACT_THR."""
    rb_lo = np.arange(N_RB) * TILE_P / RES
    rb_hi = (np.arange(N_RB) * TILE_P + (TILE_P - 1)) / RES
    ch_lo = np.arange(N_CH) * TILE_F / RES
    ch_hi = (np.arange(N_CH) * TILE_F + (TILE_F - 1)) / RES
    # distance from each curve point to each tile interval (0 if inside)
    dx = np.maximum(0.0, np.maximum(rb_lo[:, None] - cx, cx - rb_hi[:, None]))
    dy = np.maximum(0.0, np.maximum(ch_lo[:, None] - cy, cy - ch_hi[:, None]))
    # upper bound of the tile max: each step evaluated at its closest point
    d2 = dx[:, None, :] ** 2 + dy[None, :, :] ** 2  # (N_RB, N_CH, STEPS)
    bound = np.exp(-d2 / TWO_SIGMA_SQ).sum(-1) / STEPS
    return bound > ACT_THR


def _windows_for(active, R):
    """Greedy cover of each chunk's active row-blocks with windows of R
    consecutive row-blocks (bridges gaps < R). Returns [(chunk, rb_start)]."""
    wins = []
    for ch in range(N_CH):
        rbs = np.nonzero(active[:, ch])[0]
        i = 0
        while i < len(rbs):
            start = max(0, min(int(rbs[i]), N_RB - R))
            wins.append((ch, start))
            while i < len(rbs) and rbs[i] < start + R:
                i += 1
    return wins


# modeled busy us per engine for a pair copy [128,1024] (Pool cannot run
# tensor ops on this compiler, so only ACT/DVE)
_PAIR_COST = {"dve": 1.192, "act": 1.038}


def _plan_shape(active):
    """Pick even R minimizing a coarse makespan model."""
    best = None
    for R in (4, 6, 8):
        wins = _windows_for(active, R)
        C = max(1, (len(wins) + N_CORES - 1) // N_CORES)
        S = C * R
        dma = S * 0.364 + 1.0
        act = C * (0.612 + 0.107 * R + 0.185)
        busy = {"dve": 0.0, "act": act}
        for _ in range(C * (R // 2)):
            eng = min(busy, key=lambda e: busy[e] + _PAIR_COST[e])
            busy[eng] += _PAIR_COST[eng]
        score = max(dma, max(busy.values()) + 1.0, S * 0.30)
        if best is None or score < best[0]:
            best = (score, R, wins)
    return best[1], best[2]


# ------------------------------------------------------------- device build


def _build_nc(C, R):
    import concourse.mybir as mybir
    import concourse.tile as tile
    from concourse import bacc

    f32 = mybir.dt.float32
    f32r = mybir.dt.float32r
    f16 = mybir.dt.float16
    derf = mybir.ActivationFunctionType.Derivative_Erf
    cpy = mybir.ActivationFunctionType.Copy
    mult = mybir.AluOpType.mult

    nc = bacc.Bacc(
        "TRN2", target_bir_lowering=False, debug=False, num_devices=N_CORES
    )

    # bias[:, g] = K*(chunk_px/RES - cy[t]);  bias[:, C+g] = K*(win_px/RES - cx[t])
    bias_d = nc.dram_tensor("bias_in", [STEPS, 2 * C], f32, kind="ExternalInput")
    # per-core output, slot-major columns: out[p, (g*R+r)*TILE_F + c]
    out_d = nc.dram_tensor("out", [TILE_P, C * R * TILE_F], f16, kind="ExternalOutput")

    WIOTA = max(TILE_F, TILE_P * R)
    n_pairs = R // 2

    # greedy copy-engine balancing against modeled busy time (ACT starts
    # with its Gaussian workload)
    busy = {"dve": 0.0, "act": C * (0.612 + 0.107 * R + 0.185)}
    store_q = [0]  # alternate SP (HWDGE) / Pool (SWDGE) store queues

    with tile.TileContext(nc) as tc:
        with (
            tc.tile_pool(name="const", bufs=1) as const,
            tc.tile_pool(name="gyp", bufs=3) as gyp,
            tc.tile_pool(name="gxp", bufs=3) as gxp,
            tc.tile_pool(name="obuf", bufs=6) as obuf,
            tc.tile_pool(name="psmm", bufs=4, space="PSUM") as psmm,
        ):
            iota_t = const.tile([STEPS, WIOTA], f32)
            nc.gpsimd.iota(
                iota_t,
                pattern=[[1, WIOTA]],
                base=0,
                channel_multiplier=0,
                allow_small_or_imprecise_dtypes=True,
            )

            bias_t = const.tile([STEPS, 2 * C], f32)
            nc.sync.dma_start(out=bias_t, in_=bias_d.ap())

            for g in range(C):
                # single-instruction Gaussians: DerivErf(K/RES * i + bias)
                gy = gyp.tile([STEPS, TILE_F], f32r, tag="gy")
                nc.scalar.activation(
                    out=gy,
                    in_=iota_t[:, :TILE_F],
                    func=derf,
                    scale=K_GAUSS / RES,
                    bias=bias_t[:, g : g + 1],
                )
                gxw = gxp.tile([STEPS, TILE_P * R], f32r, tag="gx")
                nc.scalar.activation(
                    out=gxw,
                    in_=iota_t[:, : TILE_P * R],
                    func=derf,
                    scale=K_GAUSS / RES,
                    bias=bias_t[:, C + g : C + g + 1],
                )
                for p in range(n_pairs):
                    r0 = 2 * p
                    ps = psmm.tile([TILE_P, 2 * TILE_F], f32, tag="ps")
                    for k in (0, 1):
                        nc.tensor.matmul(
                            out=ps[:, k * TILE_F : (k + 1) * TILE_F],
                            lhsT=gxw[:, (r0 + k) * TILE_P : (r0 + k + 1) * TILE_P],
                            rhs=gy,
                            start=True,
                            stop=True,
                        )
                    ob = obuf.tile([TILE_P, 2 * TILE_F], f16, tag="ob")
                    eng = min(busy, key=lambda e: busy[e] + _PAIR_COST[e])
                    busy[eng] += _PAIR_COST[eng]
                    if eng == "act":
                        nc.scalar.activation(
                            out=ob, in_=ps, func=cpy, scale=COPY_SCALE
                        )
                    else:
                        nc.vector.tensor_scalar(
                            out=ob, in0=ps, scalar1=COPY_SCALE, scalar2=None,
                            op0=mult,
                        )
                    dst = out_d.ap()[
                        :, (g * R + r0) * TILE_F : (g * R + r0 + 2) * TILE_F
                    ]
                    if store_q[0] % 2 == 0:
                        nc.sync.dma_start(out=dst, in_=ob)
                    else:
                        nc.gpsimd.dma_start(out=dst, in_=ob)
                    store_q[0] += 1

    nc.compile()
    return nc


def _get_nc():
    return _CACHE["nc"]


# ------------------------------------------------------------------ kernel


def kernel(control_points: np.ndarray) -> np.ndarray:
    global LAST_RESULT
    from concourse.bass_utils import run_bass_kernel_spmd

    cp = np.asarray(control_points, dtype=np.float32)
    cx, cy = _bezier_xy(cp)

    active = _active_tiles(cx, cy)
    R, wins = _plan_shape(active)
    C = max(1, (len(wins) + N_CORES - 1) // N_CORES)
    wins = wins + [wins[-1]] * (N_CORES * C - len(wins))  # pad with repeats

    key = (C, R)
    if _CACHE.get("key") != key:
        _CACHE["nc"] = _build_nc(C, R)
        _CACHE["key"] = key
    nc = _CACHE["nc"]

    cxf = cx.astype(np.float32)
    cyf = cy.astype(np.float32)
    in_maps = []
    per_core = [wins[c * C : (c + 1) * C] for c in range(N_CORES)]
    kg = np.float32(K_GAUSS)
    for c in range(N_CORES):
        bias = np.empty((STEPS, 2 * C), np.float32)
        for g, (ch, rb0) in enumerate(per_core[c]):
            bias[:, g] = kg * (np.float32(ch * TILE_F) / np.float32(RES) - cyf)
            bias[:, C + g] = kg * (
                np.float32(rb0 * TILE_P) / np.float32(RES) - cxf
            )
        in_maps.append({"bias_in": np.ascontiguousarray(bias)})

    res = run_bass_kernel_spmd(
        nc, in_maps, core_ids=list(range(N_CORES)), trace=TRACE
    )
    LAST_RESULT = res

    img = np.zeros((RES, RES), np.float32)
    for c in range(N_CORES):
        arr = res.results[c]["out"]  # (TILE_P, C*R*TILE_F) fp16
        slots = (
            arr.reshape(TILE_P, C * R, TILE_F)
            .transpose(1, 0, 2)
            .astype(np.float32)
        )
        for g, (ch, rb0) in enumerate(per_core[c]):
            for r in range(R):
                rb = rb0 + r
                img[
                    rb * TILE_P : (rb + 1) * TILE_P,
                    ch * TILE_F : (ch + 1) * TILE_F,
                ] = slots[g * R + r]
    return img


# revision 10
# speedup vs baseline: 5.5275x; 1.2216x over previous
"""Cubic-Bezier Gaussian rasterizer for Trainium2 (Bass/Tile), 8-core SPMD.

Math (matches the reference):
    t = linspace(0, 1, 100);  curve = Bezier3(control_points, t)   # (2, 100)
    gx[t, i] = exp(-(curve_x[t] - i/8192)^2 / 2e-4)                # (100, 8192)
    gy[t, j] = exp(-(curve_y[t] - j/8192)^2 / 2e-4)
    out = gx^T @ gy / 100                                          # (8192, 8192)

The Gaussian tube around the curve (sigma = 0.01 = ~82 px) covers only
~15% of the 8192 x 8192 image at 128 x 512 tile granularity; everything
else is < 1e-6 (vs a Frobenius norm of ~27), far below the error that
fp16 storage already introduces (2e-4). So instead of streaming the full
256 MB f32 image (the baseline, ~104 us at the 360 GB/s DMA roofline),
the host plans the active tile set from the curve at call time and the
device computes just those tiles, in fp16. The host scatters them into a
zero image while unsharding.

Plan (host, numpy, per call; compiled program cached by shape (C, R)):
  - active tiles: dist(tile rect, curve point) bound per (row-block,
    col-chunk); threshold 1e-7 on the summed Gaussian.
  - cover each chunk's active row-blocks with windows of R consecutive
    row-blocks (greedy, bridges small gaps); pad the window list to
    8*C windows (repeats are benign: duplicate slots just rewrite the
    same correct tile).
  - per-window data = two Gaussian-center bias vectors (100 floats):
    sqrt(5000)*(chunk_px/8192 - cy[t]) and sqrt(5000)*(win_px/8192 - cx[t]).

Device pipeline per core (C groups x R slots, R even):
  ACT:  one Derivative_Erf per gy chunk [100,512] and per gx window
        [100,128R] - Derivative_Erf(u) = (2/sqrt(pi)) exp(-u^2) is a
        single-instruction Gaussian (exact to 2e-6 on TRN2); the
        (2/sqrt(pi))^2 and the 1/100 fold into the copy scale.
  PE:   R f32r matmuls per group, gxw^T @ gy -> PSUM in [128,1024] pairs
  ACT/DVE: per-pair PSUM->SBUF copies, scale pi/400, downcast to fp16
        (engine chosen greedily to balance modeled busy time; the Pool
        engine cannot run tensor ops on this compiler)
  DMA:  one store per pair (256 KB fp16, 2 KB lines), alternating
        SP/HWDGE and Pool/SWDGE issue queues; ~3 MB per core total
"""

import math

import numpy as np

RES = 8192
STEPS = 100
N_CORES = 8
TWO_SIGMA_SQ = 2e-4
K_GAUSS = math.sqrt(1.0 / TWO_SIGMA_SQ)  # sqrt(5000)
COPY_SCALE = math.pi / 4.0 / STEPS  # undo (2/sqrt(pi))^2, apply 1/STEPS

TILE_P = 128  # output tile rows (psum partition dim)
TILE_F = 512  # output tile cols (one psum bank of f32)
N_RB = RES // TILE_P  # 64 row-blocks
N_CH = RES // TILE_F  # 16 column-chunks
ACT_THR = 1e-5  # tile activity threshold on the summed-Gaussian bound
# (zeroing tiles below 1e-5 contributes ~2e-5 rel err, well under the
# 2e-4 fp16 storage floor)

_CACHE = {}

TRACE = False
LAST_RESULT = None


# ----------------------------------------------------------------- planning


def _bezier_xy(cp):
    """Cubic Bezier samples, float64, shape (2, STEPS)."""
    t = np.linspace(0.0, 1.0, STEPS)
    b = np.stack(
        [math.comb(3, k) * (1.0 - t) ** (3 - k) * t**k for k in range(4)]
    )  # (4, STEPS)
    return cp.astype(np.float64).T @ b  # (2, STEPS)


def _active_tiles(cx, cy):
    """Bool (N_RB, N_CH): tiles where the summed Gaussian can exceed ACT_THR."""
    rb_lo = np.arange(N_RB) * TILE_P / RES
    rb_hi = (np.arange(N_RB) * TILE_P + (TILE_P - 1)) / RES
    ch_lo = np.arange(N_CH) * TILE_F / RES
    ch_hi = (np.arange(N_CH) * TILE_F + (TILE_F - 1)) / RES
    # distance from each curve point to each tile interval (0 if inside)
    dx = np.maximum(0.0, np.maximum(rb_lo[:, None] - cx, cx - rb_hi[:, None]))
    dy = np.maximum(0.0, np.maximum(ch_lo[:, None] - cy, cy - ch_hi[:, None]))
    # upper bound of the tile max: each step evaluated at its closest point
    d2 = dx[:, None, :] ** 2 + dy[None, :, :] ** 2  # (N_RB, N_CH, STEPS)
    bound = np.exp(-d2 / TWO_SIGMA_SQ).sum(-1) / STEPS
    return bound > ACT_THR


def _windows_for(active, R):
    """Greedy cover of each chunk's active row-blocks with windows of R
    consecutive row-blocks (bridges gaps < R). Returns [(chunk, rb_start)]."""
    wins = []
    for ch in range(N_CH):
        rbs = np.nonzero(active[:, ch])[0]
        i = 0
        while i < len(rbs):
            start = max(0, min(int(rbs[i]), N_RB - R))
            wins.append((ch, start))
            while i < len(rbs) and rbs[i] < start + R:
                i += 1
    return wins


# modeled busy us per engine for a pair copy [128,1024] (Pool cannot run
# tensor ops on this compiler, so only ACT/DVE)
_PAIR_COST = {"dve": 1.192, "act": 1.038}


def _plan_shape(active):
    """Pick even R minimizing a coarse makespan model."""
    best = None
    for R in (4, 6, 8):
        wins = _windows_for(active, R)
        C = max(1, (len(wins) + N_CORES - 1) // N_CORES)
        S = C * R
        dma = S * 0.364 + 1.0
        act = C * (0.612 + 0.107 * R + 0.185)
        busy = {"dve": 0.0, "act": act}
        for _ in range(C * (R // 2)):
            eng = min(busy, key=lambda e: busy[e] + _PAIR_COST[e])
            busy[eng] += _PAIR_COST[eng]
        score = max(dma, max(busy.values()) + 1.0, S * 0.30)
        if best is None or score < best[0]:
            best = (score, R, wins)
    return best[1], best[2]


# ------------------------------------------------------------- device build


def _build_nc(C, R):
    import concourse.mybir as mybir
    import concourse.tile as tile
    from concourse import bacc

    f32 = mybir.dt.float32
    f32r = mybir.dt.float32r
    f16 = mybir.dt.float16
    derf = mybir.ActivationFunctionType.Derivative_Erf
    cpy = mybir.ActivationFunctionType.Copy
    mult = mybir.AluOpType.mult

    # num_devices=1: the cores never communicate (pure SPMD fan-out), and
    # a multi-device build adds a ~2.5us all-core end barrier per core.
    nc = bacc.Bacc("TRN2", target_bir_lowering=False, debug=False, num_devices=1)

    # bias[:, g] = K*(chunk_px/RES - cy[t]);  bias[:, C+g] = K*(win_px/RES - cx[t])
    bias_d = nc.dram_tensor("bias_in", [STEPS, 2 * C], f32, kind="ExternalInput")
    # per-core output, slot-major columns: out[p, (g*R+r)*TILE_F + c]
    out_d = nc.dram_tensor("out", [TILE_P, C * R * TILE_F], f16, kind="ExternalOutput")

    WIOTA = max(TILE_F, TILE_P * R)
    n_pairs = R // 2

    # greedy copy-engine balancing against modeled busy time (ACT starts
    # with its Gaussian workload)
    busy = {"dve": 0.0, "act": C * (0.612 + 0.107 * R + 0.185)}

    with tile.TileContext(nc) as tc:
        with (
            tc.tile_pool(name="const", bufs=1) as const,
            tc.tile_pool(name="gyp", bufs=3) as gyp,
            tc.tile_pool(name="gxp", bufs=3) as gxp,
            tc.tile_pool(name="obuf", bufs=6) as obuf,
            tc.tile_pool(name="psmm", bufs=4, space="PSUM") as psmm,
        ):
            # dep-free dummy activation: hoists the implicit ACT table load
            # (1.3us) off the bias-DMA critical path to t~0
            warm = const.tile([STEPS, 1], f32)
            nc.vector.memset(warm, 0.0)
            warm2 = const.tile([STEPS, 1], f32)
            nc.scalar.activation(out=warm2, in_=warm, func=derf)

            iota_t = const.tile([STEPS, WIOTA], f32)
            nc.gpsimd.iota(
                iota_t,
                pattern=[[1, WIOTA]],
                base=0,
                channel_multiplier=0,
                allow_small_or_imprecise_dtypes=True,
            )

            bias_t = const.tile([STEPS, 2 * C], f32)
            nc.sync.dma_start(out=bias_t, in_=bias_d.ap())

            for g in range(C):
                # single-instruction Gaussians: DerivErf(K/RES * i + bias)
                gy = gyp.tile([STEPS, TILE_F], f32r, tag="gy")
                nc.scalar.activation(
                    out=gy,
                    in_=iota_t[:, :TILE_F],
                    func=derf,
                    scale=K_GAUSS / RES,
                    bias=bias_t[:, g : g + 1],
                )
                gxw = gxp.tile([STEPS, TILE_P * R], f32r, tag="gx")
                nc.scalar.activation(
                    out=gxw,
                    in_=iota_t[:, : TILE_P * R],
                    func=derf,
                    scale=K_GAUSS / RES,
                    bias=bias_t[:, C + g : C + g + 1],
                )
                for p in range(n_pairs):
                    r0 = 2 * p
                    ps = psmm.tile([TILE_P, 2 * TILE_F], f32, tag="ps")
                    for k in (0, 1):
                        nc.tensor.matmul(
                            out=ps[:, k * TILE_F : (k + 1) * TILE_F],
                            lhsT=gxw[:, (r0 + k) * TILE_P : (r0 + k + 1) * TILE_P],
                            rhs=gy,
                            start=True,
                            stop=True,
                        )
                    ob = obuf.tile([TILE_P, 2 * TILE_F], f16, tag="ob")
                    eng = min(busy, key=lambda e: busy[e] + _PAIR_COST[e])
                    busy[eng] += _PAIR_COST[eng]
                    if eng == "act":
                        nc.scalar.activation(
                            out=ob, in_=ps, func=cpy, scale=COPY_SCALE
                        )
                    else:
                        nc.vector.tensor_scalar(
                            out=ob, in0=ps, scalar1=COPY_SCALE, scalar2=None,
                            op0=mult,
                        )
                    nc.sync.dma_start(
                        out=out_d.ap()[
                            :, (g * R + r0) * TILE_F : (g * R + r0 + 2) * TILE_F
                        ],
                        in_=ob,
                    )

    nc.compile()
    return nc


def _get_nc():
    return _CACHE["nc"]


# ------------------------------------------------------------------ kernel


def kernel(control_points: np.ndarray) -> np.ndarray:
    global LAST_RESULT
    from concourse.bass_utils import run_bass_kernel_spmd

    cp = np.asarray(control_points, dtype=np.float32)
    cx, cy = _bezier_xy(cp)

    active = _active_tiles(cx, cy)
    R, wins = _plan_shape(active)
    C = max(1, (len(wins) + N_CORES - 1) // N_CORES)
    wins = wins + [wins[-1]] * (N_CORES * C - len(wins))  # pad with repeats

    key = (C, R)
    if _CACHE.get("key") != key:
        _CACHE["nc"] = _build_nc(C, R)
        _CACHE["key"] = key
    nc = _CACHE["nc"]

    cxf = cx.astype(np.float32)
    cyf = cy.astype(np.float32)
    in_maps = []
    per_core = [wins[c * C : (c + 1) * C] for c in range(N_CORES)]
    kg = np.float32(K_GAUSS)
    for c in range(N_CORES):
        bias = np.empty((STEPS, 2 * C), np.float32)
        for g, (ch, rb0) in enumerate(per_core[c]):
            bias[:, g] = kg * (np.float32(ch * TILE_F) / np.float32(RES) - cyf)
            bias[:, C + g] = kg * (
                np.float32(rb0 * TILE_P) / np.float32(RES) - cxf
            )
        in_maps.append({"bias_in": np.ascontiguousarray(bias)})

    res = run_bass_kernel_spmd(
        nc, in_maps, core_ids=list(range(N_CORES)), trace=TRACE
    )
    LAST_RESULT = res

    img = np.zeros((RES, RES), np.float32)
    for c in range(N_CORES):
        arr = res.results[c]["out"]  # (TILE_P, C*R*TILE_F) fp16
        slots = (
            arr.reshape(TILE_P, C * R, TILE_F)
            .transpose(1, 0, 2)
            .astype(np.float32)
        )
        for g, (ch, rb0) in enumerate(per_core[c]):
            for r in range(R):
                rb = rb0 + r
                img[
                    rb * TILE_P : (rb + 1) * TILE_P,
                    ch * TILE_F : (ch + 1) * TILE_F,
                ] = slots[g * R + r]
    return img


# revision 12
# speedup vs baseline: 5.5478x; 1.0037x over previous
"""Cubic-Bezier Gaussian rasterizer for Trainium2 (Bass/Tile), 8-core SPMD.

Math (matches the reference):
    t = linspace(0, 1, 100);  curve = Bezier3(control_points, t)   # (2, 100)
    gx[t, i] = exp(-(curve_x[t] - i/8192)^2 / 2e-4)                # (100, 8192)
    gy[t, j] = exp(-(curve_y[t] - j/8192)^2 / 2e-4)
    out = gx^T @ gy / 100                                          # (8192, 8192)

The Gaussian tube around the curve (sigma = 0.01 = ~82 px) covers only
~15% of the 8192 x 8192 image at 128 x 512 tile granularity; everything
else is < 1e-6 (vs a Frobenius norm of ~27), far below the error that
fp16 storage already introduces (2e-4). So instead of streaming the full
256 MB f32 image (the baseline, ~104 us at the 360 GB/s DMA roofline),
the host plans the active tile set from the curve at call time and the
device computes just those tiles, in fp16. The host scatters them into a
zero image while unsharding.

Plan (host, numpy, per call; compiled program cached by shape (C, R)):
  - active tiles: dist(tile rect, curve point) bound per (row-block,
    col-chunk); threshold 1e-7 on the summed Gaussian.
  - cover each chunk's active row-blocks with windows of R consecutive
    row-blocks (greedy, bridges small gaps); pad the window list to
    8*C windows (repeats are benign: duplicate slots just rewrite the
    same correct tile).
  - per-window data = two Gaussian-center bias vectors (100 floats):
    sqrt(5000)*(chunk_px/8192 - cy[t]) and sqrt(5000)*(win_px/8192 - cx[t]).

Device pipeline per core (C groups x R slots, R even):
  ACT:  one Derivative_Erf per gy chunk [100,512] and per gx window
        [100,128R] - Derivative_Erf(u) = (2/sqrt(pi)) exp(-u^2) is a
        single-instruction Gaussian (exact to 2e-6 on TRN2); the
        (2/sqrt(pi))^2 and the 1/100 fold into the copy scale.
  PE:   R f32r matmuls per group, gxw^T @ gy -> PSUM in [128,1024] pairs
  ACT/DVE: per-pair PSUM->SBUF copies, scale pi/400, downcast to fp16
        (engine chosen greedily to balance modeled busy time; the Pool
        engine cannot run tensor ops on this compiler)
  DMA:  one store per pair (256 KB fp16, 2 KB lines), alternating
        SP/HWDGE and Pool/SWDGE issue queues; ~3 MB per core total
"""

import math

import numpy as np

RES = 8192
STEPS = 100
N_CORES = 8
TWO_SIGMA_SQ = 2e-4
K_GAUSS = math.sqrt(1.0 / TWO_SIGMA_SQ)  # sqrt(5000)
COPY_SCALE = math.pi / 4.0 / STEPS  # undo (2/sqrt(pi))^2, apply 1/STEPS

TILE_P = 128  # output tile rows (psum partition dim)
TILE_F = 512  # output tile cols (one psum bank of f32)
N_RB = RES // TILE_P  # 64 row-blocks
N_CH = RES // TILE_F  # 16 column-chunks
ACT_THR = 1e-5  # tile activity threshold on the summed-Gaussian bound
# (zeroing tiles below 1e-5 contributes ~2e-5 rel err, well under the
# 2e-4 fp16 storage floor)

_CACHE = {}

TRACE = False
LAST_RESULT = None


# ----------------------------------------------------------------- planning


def _bezier_xy(cp):
    """Cubic Bezier samples, float64, shape (2, STEPS)."""
    t = np.linspace(0.0, 1.0, STEPS)
    b = np.stack(
        [math.comb(3, k) * (1.0 - t) ** (3 - k) * t**k for k in range(4)]
    )  # (4, STEPS)
    return cp.astype(np.float64).T @ b  # (2, STEPS)


def _active_tiles(cx, cy):
    """Bool (N_RB, N_CH): tiles where the summed Gaussian can exceed ACT_THR."""
    rb_lo = np.arange(N_RB) * TILE_P / RES
    rb_hi = (np.arange(N_RB) * TILE_P + (TILE_P - 1)) / RES
    ch_lo = np.arange(N_CH) * TILE_F / RES
    ch_hi = (np.arange(N_CH) * TILE_F + (TILE_F - 1)) / RES
    # distance from each curve point to each tile interval (0 if inside)
    dx = np.maximum(0.0, np.maximum(rb_lo[:, None] - cx, cx - rb_hi[:, None]))
    dy = np.maximum(0.0, np.maximum(ch_lo[:, None] - cy, cy - ch_hi[:, None]))
    # upper bound of the tile max: each step evaluated at its closest point
    d2 = dx[:, None, :] ** 2 + dy[None, :, :] ** 2  # (N_RB, N_CH, STEPS)
    bound = np.exp(-d2 / TWO_SIGMA_SQ).sum(-1) / STEPS
    return bound > ACT_THR


def _windows_for(active, R):
    """Greedy cover of each chunk's active row-blocks with windows of R
    consecutive row-blocks (bridges gaps < R). Returns [(chunk, rb_start)]."""
    wins = []
    for ch in range(N_CH):
        rbs = np.nonzero(active[:, ch])[0]
        i = 0
        while i < len(rbs):
            start = max(0, min(int(rbs[i]), N_RB - R))
            wins.append((ch, start))
            while i < len(rbs) and rbs[i] < start + R:
                i += 1
    return wins


# modeled busy us per engine for a pair copy [128,1024] (Pool cannot run
# tensor ops on this compiler, so only ACT/DVE)
_PAIR_COST = {"dve": 1.192, "act": 1.038}


def _plan_shape(active):
    """Pick even R minimizing a coarse makespan model."""
    best = None
    for R in (4, 6, 8):
        wins = _windows_for(active, R)
        C = max(1, (len(wins) + N_CORES - 1) // N_CORES)
        S = C * R
        dma = S * 0.364 + 1.0
        act = C * (0.612 + 0.107 * R + 0.185)
        busy = {"dve": 0.0, "act": act}
        for _ in range(C * (R // 2)):
            eng = min(busy, key=lambda e: busy[e] + _PAIR_COST[e])
            busy[eng] += _PAIR_COST[eng]
        score = max(dma, max(busy.values()) + 1.0, S * 0.30)
        if best is None or score < best[0]:
            best = (score, R, wins)
    return best[1], best[2]


# ------------------------------------------------------------- device build


def _build_nc(C, R):
    import concourse.mybir as mybir
    import concourse.tile as tile
    from concourse import bacc

    f32 = mybir.dt.float32
    f32r = mybir.dt.float32r
    f16 = mybir.dt.float16
    derf = mybir.ActivationFunctionType.Derivative_Erf
    cpy = mybir.ActivationFunctionType.Copy
    mult = mybir.AluOpType.mult

    # num_devices=1: the cores never communicate (pure SPMD fan-out), and
    # a multi-device build adds a ~2.5us all-core end barrier per core.
    nc = bacc.Bacc("TRN2", target_bir_lowering=False, debug=False, num_devices=1)

    # bias[:, g] = K*(chunk_px/RES - cy[t]);  bias[:, C+g] = K*(win_px/RES - cx[t])
    bias_d = nc.dram_tensor("bias_in", [STEPS, 2 * C], f32, kind="ExternalInput")
    # per-core output, slot-major columns: out[p, (g*R+r)*TILE_F + c]
    out_d = nc.dram_tensor("out", [TILE_P, C * R * TILE_F], f16, kind="ExternalOutput")

    WIOTA = max(TILE_F, TILE_P * R)
    n_pairs = R // 2

    # greedy copy-engine balancing against modeled busy time (ACT starts
    # with its Gaussian workload)
    busy = {"dve": 0.0, "act": C * (0.612 + 0.107 * R + 0.185)}

    with tile.TileContext(nc) as tc:
        with (
            tc.tile_pool(name="const", bufs=1) as const,
            tc.tile_pool(name="gyp", bufs=C) as gyp,
            tc.tile_pool(name="gxp", bufs=C) as gxp,
            tc.tile_pool(name="obuf", bufs=6) as obuf,
            tc.tile_pool(name="psmm", bufs=3, space="PSUM") as psmm,
            tc.tile_pool(name="pswarm", bufs=1, space="PSUM") as pswarm,
        ):
            # dep-free dummy activation: hoists the implicit ACT table load
            # (1.3us) off the bias-DMA critical path to t~0
            warm = const.tile([STEPS, 1], f32)
            nc.vector.memset(warm, 0.0)
            warm2 = const.tile([STEPS, 1], f32)
            nc.scalar.activation(out=warm2, in_=warm, func=derf)

            # PE p-state pre-ramp: a few dep-free matmuls during the bias
            # DMA window start the PE "continuous run" clock early, so the
            # real matmuls run at mid/full speed instead of 0.65 GHz
            w0 = const.tile([STEPS, TILE_F], f32)
            nc.vector.memset(w0, 0.0)
            wmm = const.tile([STEPS, TILE_F], f32r)
            nc.vector.tensor_copy(out=wmm, in_=w0)
            ps_w = pswarm.tile([TILE_P, TILE_F], f32)
            for _ in range(3):
                nc.tensor.matmul(
                    out=ps_w, lhsT=wmm[:, :TILE_P], rhs=wmm, start=True, stop=True
                )

            iota_t = const.tile([STEPS, WIOTA], f32)
            nc.gpsimd.iota(
                iota_t,
                pattern=[[1, WIOTA]],
                base=0,
                channel_multiplier=0,
                allow_small_or_imprecise_dtypes=True,
            )

            bias_t = const.tile([STEPS, 2 * C], f32)
            nc.sync.dma_start(out=bias_t, in_=bias_d.ap())

            # all Gaussians first: keeps ACT SEQ free of copy stalls, so
            # every group's gy/gxw is ready as early as possible
            gys, gxws = [], []
            for g in range(C):
                gy = gyp.tile([STEPS, TILE_F], f32r, tag="gy")
                nc.scalar.activation(
                    out=gy,
                    in_=iota_t[:, :TILE_F],
                    func=derf,
                    scale=K_GAUSS / RES,
                    bias=bias_t[:, g : g + 1],
                )
                gys.append(gy)
                gxw = gxp.tile([STEPS, TILE_P * R], f32r, tag="gx")
                nc.scalar.activation(
                    out=gxw,
                    in_=iota_t[:, : TILE_P * R],
                    func=derf,
                    scale=K_GAUSS / RES,
                    bias=bias_t[:, C + g : C + g + 1],
                )
                gxws.append(gxw)

            for g in range(C):
                for p in range(n_pairs):
                    r0 = 2 * p
                    ps = psmm.tile([TILE_P, 2 * TILE_F], f32, tag="ps")
                    for k in (0, 1):
                        nc.tensor.matmul(
                            out=ps[:, k * TILE_F : (k + 1) * TILE_F],
                            lhsT=gxws[g][
                                :, (r0 + k) * TILE_P : (r0 + k + 1) * TILE_P
                            ],
                            rhs=gys[g],
                            start=True,
                            stop=True,
                        )
                    ob = obuf.tile([TILE_P, 2 * TILE_F], f16, tag="ob")
                    eng = min(busy, key=lambda e: busy[e] + _PAIR_COST[e])
                    busy[eng] += _PAIR_COST[eng]
                    if eng == "act":
                        nc.scalar.activation(
                            out=ob, in_=ps, func=cpy, scale=COPY_SCALE
                        )
                    else:
                        nc.vector.tensor_scalar(
                            out=ob, in0=ps, scalar1=COPY_SCALE, scalar2=None,
                            op0=mult,
                        )
                    nc.sync.dma_start(
                        out=out_d.ap()[
                            :, (g * R + r0) * TILE_F : (g * R + r0 + 2) * TILE_F
                        ],
                        in_=ob,
                    )

    nc.compile()
    return nc


def _get_nc():
    return _CACHE["nc"]


# ------------------------------------------------------------------ kernel


def kernel(control_points: np.ndarray) -> np.ndarray:
    global LAST_RESULT
    from concourse.bass_utils import run_bass_kernel_spmd

    cp = np.asarray(control_points, dtype=np.float32)
    cx, cy = _bezier_xy(cp)

    active = _active_tiles(cx, cy)
    R, wins = _plan_shape(active)
    C = max(1, (len(wins) + N_CORES - 1) // N_CORES)
    wins = wins + [wins[-1]] * (N_CORES * C - len(wins))  # pad with repeats

    key = (C, R)
    if _CACHE.get("key") != key:
        _CACHE["nc"] = _build_nc(C, R)
        _CACHE["key"] = key
    nc = _CACHE["nc"]

    cxf = cx.astype(np.float32)
    cyf = cy.astype(np.float32)
    in_maps = []
    per_core = [wins[c * C : (c + 1) * C] for c in range(N_CORES)]
    kg = np.float32(K_GAUSS)
    for c in range(N_CORES):
        bias = np.empty((STEPS, 2 * C), np.float32)
        for g, (ch, rb0) in enumerate(per_core[c]):
            bias[:, g] = kg * (np.float32(ch * TILE_F) / np.float32(RES) - cyf)
            bias[:, C + g] = kg * (
                np.float32(rb0 * TILE_P) / np.float32(RES) - cxf
            )
        in_maps.append({"bias_in": np.ascontiguousarray(bias)})

    res = run_bass_kernel_spmd(
        nc, in_maps, core_ids=list(range(N_CORES)), trace=TRACE
    )
    LAST_RESULT = res

    img = np.zeros((RES, RES), np.float32)
    for c in range(N_CORES):
        arr = res.results[c]["out"]  # (TILE_P, C*R*TILE_F) fp16
        slots = (
            arr.reshape(TILE_P, C * R, TILE_F)
            .transpose(1, 0, 2)
            .astype(np.float32)
        )
        for g, (ch, rb0) in enumerate(per_core[c]):
            for r in range(R):
                rb = rb0 + r
                img[
                    rb * TILE_P : (rb + 1) * TILE_P,
                    ch * TILE_F : (ch + 1) * TILE_F,
                ] = slots[g * R + r]
    return img
